# revision 39
# baseline (speedup 1.0000x reference)
"""Trainium2 Bass kernel for nn_DilatedGCN (gnn_message_passing).

Math (derived from the reference):
  feats F = X @ W_mlp + b_mlp                  [N, B, T, D]
  scores = concat([F[src], F[dst]]) @ W_attn + b_attn
  Per-destination-segment softmax over the DEG=8 incoming edges.
  The dst-side term is constant within a segment, so it cancels in the
  softmax; max-subtraction is unnecessary in f32.  Hence with
     S  = F @ W_attn[:D]        (per node)
     ES = exp(S)/8              (per node; /8 keeps fp8e4 in range and
                                 cancels in num/den)
     G  = ES * F                (per node)
  each dilation graph k is a segment-sum over incoming edges:
     gcn_k[n] = (sum_j G[src_j]) / (sum_j ES[src_j])
  out = leaky_relu(sum_k w_k * gcn_k, 0.01) + X

Key idea vs the gather-based variant: dst = repeat(arange(N), 8), so
gather+segment-sum == dense matmul with the (tiny-valued, exact-in-fp8)
edge-count matrix A_k[dst, src]:
     [den | num] = A_k @ [ES | G]
A DMA row-gather is HBM-latency-bound; the dense matmul replaces it with
fp8 DoubleRow TensorE work plus 12 MB of sequential A-tile loads that
overlap compute.

Distribution: data-parallel over the 48 (b, t) pairs -> 6 per core.
Per core: the MLP runs per node-s-block (6 matmuls x 2 halves into two
PSUM banks, one batched Exp on Act, one batched mult on DVE) writing the
node tables H = [ES | G] (fp8, [128, 16 s-blocks, 768]) in SBUF; per
(ot, k) 8+8 DoubleRow matmuls (contraction 256/instr) produce den/num in
PSUM; the epilogue is bf16 and spread across engines: DVE reciprocal,
Pool folds w_k into num, DVE multiply/accumulate, Act Lrelu(alpha=.01),
HWDGE residual-accumulate DMA + store.
"""

import os as _os
import shutil as _shutil

import numpy as np

# The libneuronxla on-disk NEFF cache keys on the XLA module NAME, not its
# content (libncc.py: cache_key = file_prefix.split('_')[-1]), so two
# different bass programs with the same jit callsite collide and the second
# silently runs the first one's stale NEFF. Disable the cache and clear any
# poisoned entries. One recompile is cheap; a wrong NEFF is not.
_shutil.rmtree(_os.path.expanduser("~/.neuron-compile-cache"),
               ignore_errors=True)

B, N, T, C, D, K, DEG = 4, 2000, 12, 64, 64, 3, 8
E = N * DEG
NCORES = 8
BT = B * T              # 48
SPC = BT // NCORES      # 6 (b,t) slots per core
M = SPC * D             # 384 channels per node per core
NSB = 16                # node s-blocks of 128 (2000 -> 2048 padded)
NP = 128 * NSB          # 2048 padded nodes
NCH = NSB * SPC         # 96 MLP chunks of 128 nodes x 1 slot
LN8 = float(np.log(8.0))

_CACHE = {}


def _build_program(kstage=None, ksub=None, rep_all=1):
    import concourse.bacc as bacc
    import concourse.mybir as mybir
    from concourse.tile import TileContext
    from contextlib import ExitStack

    kstage, ksub = _resolve_kargs(kstage, ksub)

    dt = mybir.dt
    nc = bacc.Bacc("TRN2")

    # Shape-encodes (rep_all, kstage, ksub) so every program variant gets a
    # distinct XLA fingerprint: the libneuronxla NEFF disk cache can collide
    # across programs whose external I/O signatures match, silently running
    # a stale NEFF. Consumed by a single tiny DMA so it can't be pruned.
    reptag = nc.dram_tensor("reptag", [1, _tagw(kstage, ksub, rep_all)],
                            dt.float32, kind="ExternalInput")
    xT1 = nc.dram_tensor("xT1", [C + 1, NCH * 128], dt.bfloat16,
                         kind="ExternalInput")
    w2cat = nc.dram_tensor("w2cat", [C + 1, 2 * D], dt.bfloat16,
                           kind="ExternalInput")
    # A^T tiles: atiles[ot, p, (k*16+s)*128 + c] = #edges(k, dst=128*ot+c,
    # src=128*s+p); counts <= 8 are exact in fp8e4.  ot-major so the first
    # dst-tile's lhsT data (all 3 graphs) lands in SBUF within ~2 us.
    atiles = nc.dram_tensor("atiles", [NSB, 128, K * NSB * 128], dt.float8e4,
                            kind="ExternalInput")
    wkcol = nc.dram_tensor("wkcol", [128, K], dt.float32, kind="ExternalInput")
    x_rows = nc.dram_tensor("x_rows", [N, M], dt.float32, kind="ExternalInput")
    out_rows = nc.dram_tensor("out_rows", [N, M], dt.float32,
                              kind="ExternalOutput")

    with TileContext(nc) as tc, ExitStack() as ctx:
        from concourse.library_config import mlp
        nc.gpsimd.load_library(mlp)
        const = ctx.enter_context(tc.tile_pool(name="const", bufs=1))
        sc = ctx.enter_context(tc.tile_pool(name="scratch", bufs=4))
        ep = ctx.enter_context(tc.tile_pool(name="epi", bufs=2))
        ps8 = ctx.enter_context(tc.tile_pool(name="ps8", bufs=2,
                                             space="PSUM"))

        tag_sb = const.tile([1, _tagw(kstage, ksub, rep_all)], dt.float32)
        nc.scalar.dma_start(tag_sb[:], reptag[:])
        xsb = const.tile([C + 1, NCH * 128], dt.bfloat16, name="xsb")

        for _rep in range(rep_all):
            _kernel_body(nc, tc, dt, mybir, kstage, ksub,
                         const, sc, ep, ps8, xsb,
                         xT1, w2cat, atiles, wkcol, x_rows, out_rows,
                         first=(_rep == 0), last=(_rep == rep_all - 1))

    nc.compile()
    return nc


def _kernel_body(nc, tc, dt, mybir, KSTAGE, KSUB,
                 const, sc, ep, ps8, xsb,
                 xT1, w2cat, atiles, wkcol, x_rows, out_rows,
                 first=True, last=True):
    AF = mybir.ActivationFunctionType
    ALU = mybir.AluOpType
    DR = mybir.MatmulPerfMode.DoubleRow

    # ---------------- loads ----------------
    # xT1 split per s-block: first MLP matmul can start ~3 us in.
    wq = SPC * 128
    if first:
        for s in range(NSB):
            nc.sync.dma_start(xsb[:, s * wq:(s + 1) * wq],
                              xT1[:, s * wq:(s + 1) * wq])
    w2_sb = sc.tile([C + 1, 2 * D], dt.bfloat16, tag="w2")
    nc.scalar.dma_start(w2_sb[:], w2cat[:])
    wk_sb = sc.tile([128, K], dt.float32, tag="wk")
    nc.scalar.dma_start(wk_sb[:], wkcol[:])
    bias_t = sc.tile([128, 1], dt.float32, tag="bias")
    nc.gpsimd.memset(bias_t[:], -LN8)

    # A^T slab: one 768 KB DMA per dst-tile (covers all 3 graphs).
    # Slabs 0-3 load upfront; the rest are JIT-prefetched inside the main
    # loop so the DMA device isn't saturated while epilogue I/O needs it.
    at_all = const.tile([128, NSB, K, NSB, 128], dt.float8e4)

    def load_at(ot):
        if "g" in KSUB:
            nc.sync.dma_start(
                at_all[:, ot].rearrange("p k s c -> p (k s c)"), atiles[ot])

    for ot in range(4):
        load_at(ot)
    if "g" not in KSUB:
        nc.gpsimd.memset(at_all[:], 0.125)

    # residual slabs prefetched into SBUF (avoids CCE-accumulate DMA parks
    # on the Pool queue); Pool does the add as a compute op instead.
    xr_tiles = {}

    def load_xr(ot):
        pv2 = 128 if ot < NSB - 1 else N - 128 * (NSB - 1)
        xr = ep.tile([128, M], dt.float32, tag="xr", bufs=4, name=f"xr{ot}")
        nc.sync.dma_start(xr[:pv2, :], x_rows[128 * ot:128 * ot + pv2, :])
        xr_tiles[ot] = xr

    # node tables: H[p, s, 0:384] = ES, H[p, s, 384:768] = G  (node 128s+p)
    # bufs=2 so rep i+1's prologue H writes overlap rep i's main loop
    # (matters for steady-state throughput when the body is replicated).
    H = ep.tile([128, NSB, 2 * M], dt.float8e4, tag="H", bufs=2, name="H")

    # ---------------- prologue: MLP -> H in SBUF, batched per s-block ------
    for s in range(NSB):
        sps = ps8.tile([128, M], dt.float32, tag="ps", bufs=8)
        fps = ps8.tile([128, M], dt.float32, tag="ps", bufs=8)
        for t in range(SPC):
            ci = s * SPC + t
            lt = xsb[:, 128 * ci:128 * (ci + 1)]
            nc.tensor.matmul(out=sps[:, D * t:D * (t + 1)], lhsT=lt,
                             rhs=w2_sb[:, D:], start=True, stop=True)
        for t in range(SPC):
            ci = s * SPC + t
            lt = xsb[:, 128 * ci:128 * (ci + 1)]
            nc.tensor.matmul(out=fps[:, D * t:D * (t + 1)], lhsT=lt,
                             rhs=w2_sb[:, :D], start=True, stop=True)
        esv = H[:, s, :M]
        nc.scalar.activation(esv, sps[:], AF.Exp, bias=bias_t[:])
        nc.vector.tensor_tensor(H[:, s, M:], fps[:], esv, op=ALU.mult)

    load_xr(0)
    load_xr(1)

    # Deferred per-ot tail: leaky (DVE) + residual (Pool) + store (SP),
    # emitted AFTER the next ot's k=0 drain ops so the DVE-queue park on
    # the Pool-produced sum never blocks a PSUM-draining recip/stt.
    def emit_tail(pend):
        ot0, pv0, acc0 = pend
        lr = ep.tile([128, M], dt.bfloat16, tag="lr", bufs=3, name="lr")
        nc.vector.scalar_tensor_tensor(lr[:pv0, :], acc0[:pv0, :], 0.01,
                                       acc0[:pv0, :], op0=ALU.mult,
                                       op1=ALU.max)
        ott = ep.tile([128, M], dt.float32, tag="out", bufs=3, name="ott")
        if KSTAGE >= "3":
            nc.gpsimd.tensor_tensor(ott[:pv0, :], lr[:pv0, :],
                                    xr_tiles[ot0][:pv0, :], op=ALU.add)
        else:
            nc.vector.tensor_copy(ott[:pv0, :], lr[:pv0, :])
        nc.sync.dma_start(out_rows[128 * ot0:128 * ot0 + pv0, :],
                          ott[:pv0, :])

    pend = None

    # ------------- main: [den|num] = A_k^T @ [ES|G], fused epilogue -------
    for ot in range(NSB if KSTAGE >= "1" else 0):
        pv = 128 if ot < NSB - 1 else N - 128 * (NSB - 1)
        if ot + 4 < NSB:
            load_at(ot + 4)
        if ot + 2 < NSB:
            load_xr(ot + 2)
        if ot == 8 and not last:
            # issue the NEXT rep's xsb loads now: by the rep boundary the
            # data is resident, so the next prologue starts with no PE gap
            # (and rides the warm pstate ramp).  WAR vs this rep's prologue
            # reads is long resolved.
            for s in range(NSB):
                nc.sync.dma_start(xsb[:, s * wq:(s + 1) * wq],
                                  xT1[:, s * wq:(s + 1) * wq])
        parts = []
        for k in range(K):
            if "m" not in KSUB:
                continue
            denp = ps8.tile([128, M], dt.float32, tag="ps", bufs=8)
            nump = ps8.tile([128, M], dt.float32, tag="ps", bufs=8)
            for s2 in range(NSB // 2):
                lt = at_all[:, ot, k, 2 * s2:2 * s2 + 2, :]
                nc.tensor.matmul(out=denp[:], lhsT=lt,
                                 rhs=H[:, 2 * s2:2 * s2 + 2, :M],
                                 start=(s2 == 0), stop=(s2 == NSB // 2 - 1),
                                 perf_mode=DR)
                nc.tensor.matmul(out=nump[:], lhsT=lt,
                                 rhs=H[:, 2 * s2:2 * s2 + 2, M:],
                                 start=(s2 == 0), stop=(s2 == NSB // 2 - 1),
                                 perf_mode=DR)
            if "e" not in KSUB:
                continue
            # epilogue part 1, all on DVE so the PSUM banks drain with zero
            # cross-engine hops: recip frees den; the fused stt
            # tmp_k = (num * w_k) * rden frees num.
            rdenb = ep.tile([128, M], dt.bfloat16, tag="rden", bufs=3)
            with nc.allow_low_precision(reason="bf16 softmax recip; "
                                        "tol 2e-2 >> bf16 eps"):
                nc.vector.reciprocal(out=rdenb[:pv, :], in_=denp[:pv, :])
            tmp = ep.tile([128, M], dt.bfloat16, tag=f"tmp{k}", bufs=2)
            nc.vector.scalar_tensor_tensor(tmp[:pv, :], nump[:pv, :],
                                           wk_sb[:pv, k:k + 1], rdenb[:pv, :],
                                           op0=ALU.mult, op1=ALU.mult)
            parts.append(tmp)
            if k == 0 and pend is not None:
                emit_tail(pend)
                pend = None
        if KSTAGE < "2" or not parts:
            continue
        # part 2 on Pool (all-SBUF bf16): both adds; the sum feeds the
        # deferred tail above.
        p01 = ep.tile([128, M], dt.bfloat16, tag="p01", bufs=2)
        nc.gpsimd.tensor_tensor(p01[:pv, :], parts[0][:pv, :],
                                parts[1][:pv, :], op=ALU.add)
        accv = ep.tile([128, M], dt.bfloat16, tag="acc", bufs=2)
        nc.gpsimd.tensor_tensor(accv[:pv, :], p01[:pv, :],
                                parts[2][:pv, :], op=ALU.add)
        pend = (ot, pv, accv)
    if pend is not None:
        emit_tail(pend)


def _resolve_kargs(kstage, ksub):
    import os
    if kstage is None:
        kstage = os.environ.get("KSTAGE", "3")
    if ksub is None:
        ksub = os.environ.get("KSUB", "gmeL")
    return kstage, ksub


def _tagw(kstage, ksub, rep_all):
    return 2 + rep_all * 8 + (ord(kstage) - ord("0")) + len(ksub) * 131


def _get_program(kstage=None, ksub=None, rep_all=1):
    key = ("nc", kstage, ksub, rep_all)
    if key not in _CACHE:
        _CACHE[key] = _build_program(kstage, ksub, rep_all)
    return _CACHE[key]


def _prep_inputs(input_feature, W_mlp, b_mlp, W_attn, b_attn, weight, edges):
    import ml_dtypes
    bf16 = ml_dtypes.bfloat16
    fp8 = ml_dtypes.float8_e4m3

    X = np.asarray(input_feature, dtype=np.float32)
    src = np.asarray(edges)[:, 0, :].astype(np.int64)
    dst = np.asarray(edges)[:, 1, :].astype(np.int64)
    assert src.min() >= 0 and src.max() < N
    assert dst.min() >= 0 and dst.max() < N

    A65 = np.concatenate([np.asarray(W_mlp, np.float32),
                          np.asarray(b_mlp, np.float32)[None, :]], axis=0)
    Wa = np.asarray(W_attn, np.float32)[:D, :]
    w2cat_h = np.ascontiguousarray(
        np.concatenate([A65, A65 @ Wa], axis=1).astype(bf16))  # [65, 128]

    # edge-count tiles: at_h[ot, p, k, s, c] = #edges(k, dst=128ot+c, src=128s+p)
    counts = np.zeros((K, NP, NP), np.uint8)
    kk = np.repeat(np.arange(K), E)
    np.add.at(counts, (kk, src.reshape(-1), dst.reshape(-1)), 1)
    at_h = np.ascontiguousarray(
        counts.reshape(K, NSB, 128, NSB, 128).transpose(3, 2, 0, 1, 4)
        .reshape(NSB, 128, K * NSB * 128).astype(fp8))

    wk = np.asarray(weight, np.float32).reshape(K)
    wkcol_h = np.ascontiguousarray(
        np.broadcast_to(wk[None, :], (128, K)).astype(np.float32))

    # per-core slices: slot = b*T + t; core c owns slots [6c, 6c+6)
    Xn = np.transpose(X, (1, 0, 2, 3)).reshape(N, BT, C)
    in_maps = []
    for c in range(NCORES):
        Xloc = Xn[:, SPC * c:SPC * (c + 1), :]                   # [N, 6, C]
        x_rows_h = np.ascontiguousarray(Xloc.reshape(N, M))
        Xpad = np.zeros((NP, SPC, C), np.float32)
        Xpad[:N] = Xloc
        xT1_h = np.empty((C + 1, NCH * 128), dtype=bf16)
        # col (s*SPC+t)*128 + i -> node 128s+i, slot t
        xT1_h[:C] = (Xpad.reshape(NSB, 128, SPC, C)
                     .transpose(3, 0, 2, 1).reshape(C, NCH * 128).astype(bf16))
        xT1_h[C] = np.asarray(1.0, dtype=bf16)
        in_maps.append({
            "xT1": np.ascontiguousarray(xT1_h),
            "w2cat": w2cat_h,
            "atiles": at_h,
            "wkcol": wkcol_h,
            "x_rows": x_rows_h,
        })
    return in_maps


def _assemble_output(results):
    out_all = np.empty((N, BT, C), dtype=np.float32)
    for c in range(NCORES):
        out_all[:, SPC * c:SPC * (c + 1), :] = \
            results[c]["out_rows"].reshape(N, SPC, C)
    return np.ascontiguousarray(
        out_all.reshape(N, B, T, C).transpose(1, 0, 2, 3))


def kernel(input_feature, W_mlp, b_mlp, W_attn, b_attn, weight, edges,
           _trace=False, **trace_kwargs):
    from concourse.bass_utils import run_bass_kernel_spmd

    in_maps = _prep_inputs(input_feature, W_mlp, b_mlp, W_attn, b_attn,
                           weight, edges)
    nc = _get_program()
    kstage, ksub = _resolve_kargs(None, None)
    tag = np.zeros((1, _tagw(kstage, ksub, 1)), np.float32)
    in_maps = [{**m, "reptag": tag} for m in in_maps]
    res = run_bass_kernel_spmd(nc, in_maps, list(range(NCORES)),
                               trace=_trace, **trace_kwargs)
    out = _assemble_output(res.results)
    if _trace:
        return out, res
    return out


# revision 44
# speedup vs baseline: 1.3452x; 1.3452x over previous
"""Trainium2 Bass kernel for nn_DilatedGCN (gnn_message_passing).

Math (derived from the reference):
  feats F = X @ W_mlp + b_mlp                  [N, B, T, D]
  scores = concat([F[src], F[dst]]) @ W_attn + b_attn
  Per-destination-segment softmax over the DEG=8 incoming edges.
  The dst-side term is constant within a segment, so it cancels in the
  softmax; max-subtraction is unnecessary in f32.  Hence with
     S  = F @ W_attn[:D]        (per node)
     ES = exp(S)/8              (per node; /8 keeps fp8e4 in range and
                                 cancels in num/den)
     G  = ES * F                (per node)
  each dilation graph k is a segment-sum over incoming edges:
     gcn_k[n] = (sum_j G[src_j]) / (sum_j ES[src_j])
  out = leaky_relu(sum_k w_k * gcn_k, 0.01) + X

Key idea vs the gather-based variant: dst = repeat(arange(N), 8), so
gather+segment-sum == dense matmul with the (tiny-valued, exact-in-fp8)
edge-count matrix A_k[dst, src]:
     [den | num] = A_k @ [ES | G]
A DMA row-gather is HBM-latency-bound; the dense matmul replaces it with
fp8 DoubleRow TensorE work plus 12 MB of sequential A-tile loads that
overlap compute.

Distribution: data-parallel over the 48 (b, t) pairs -> 6 per core.
Per core: the MLP runs per node-s-block (6 matmuls x 2 halves into two
PSUM banks, one batched Exp on Act, one batched mult on DVE) writing the
node tables H = [ES | G] (fp8, [128, 16 s-blocks, 768]) in SBUF; per
(ot, k) 8+8 DoubleRow matmuls (contraction 256/instr) produce den/num
in a single 8-bank rotating PSUM pool.  The epilogue drains PSUM with
minimal per-engine latency: DVE reciprocal_approx_fast frees den (the
bit-exact nc.vector.reciprocal is ~6 cycles/elem on HW -- never use it
here), Act frees num via Copy-with-scale folding w_k, DVE multiplies
num*rden in bf16, Pool sums the K=3 graph terms and adds the
SBUF-prefetched residual, leaky_relu(x,.01)=max(x,.01x) is one DVE stt,
stores go out on the SP queue.  A-slab and residual DMAs are
JIT-prefetched inside the ot loop so the DMA engines are never
saturated while epilogue I/O needs them; engine queues never carry DMAs
that would park in front of compute (parked DMAs block the whole
in-order queue).  HW-legality notes: Pool/GPSIMD cannot touch PSUM and
cannot run TensorScalarPtr; those constraints shaped the engine split.
"""

import os as _os
import shutil as _shutil

import numpy as np

# The libneuronxla on-disk NEFF cache keys on the XLA module NAME, not its
# content (libncc.py: cache_key = file_prefix.split('_')[-1]), so two
# different bass programs with the same jit callsite collide and the second
# silently runs the first one's stale NEFF. Disable the cache and clear any
# poisoned entries. One recompile is cheap; a wrong NEFF is not.
_shutil.rmtree(_os.path.expanduser("~/.neuron-compile-cache"),
               ignore_errors=True)

B, N, T, C, D, K, DEG = 4, 2000, 12, 64, 64, 3, 8
E = N * DEG
NCORES = 8
BT = B * T              # 48
SPC = BT // NCORES      # 6 (b,t) slots per core
M = SPC * D             # 384 channels per node per core
NSB = 16                # node s-blocks of 128 (2000 -> 2048 padded)
NP = 128 * NSB          # 2048 padded nodes
NCH = NSB * SPC         # 96 MLP chunks of 128 nodes x 1 slot
LN8 = float(np.log(8.0))

_CACHE = {}


def _build_program(kstage=None, ksub=None, rep_all=1):
    import concourse.bacc as bacc
    import concourse.mybir as mybir
    from concourse.tile import TileContext
    from contextlib import ExitStack

    kstage, ksub = _resolve_kargs(kstage, ksub)

    dt = mybir.dt
    nc = bacc.Bacc("TRN2")

    # Shape-encodes (rep_all, kstage, ksub) so every program variant gets a
    # distinct XLA fingerprint: the libneuronxla NEFF disk cache can collide
    # across programs whose external I/O signatures match, silently running
    # a stale NEFF. Consumed by a single tiny DMA so it can't be pruned.
    reptag = nc.dram_tensor("reptag", [1, _tagw(kstage, ksub, rep_all)],
                            dt.float32, kind="ExternalInput")
    xT1 = nc.dram_tensor("xT1", [C + 1, NCH * 128], dt.bfloat16,
                         kind="ExternalInput")
    w2cat = nc.dram_tensor("w2cat", [C + 1, 2 * D], dt.bfloat16,
                           kind="ExternalInput")
    # A^T tiles: atiles[ot, p, (k*16+s)*128 + c] = #edges(k, dst=128*ot+c,
    # src=128*s+p); counts <= 8 are exact in fp8e4.  ot-major so the first
    # dst-tile's lhsT data (all 3 graphs) lands in SBUF within ~2 us.
    atiles = nc.dram_tensor("atiles", [NSB, 128, K * NSB * 128], dt.float8e4,
                            kind="ExternalInput")
    wkcol = nc.dram_tensor("wkcol", [128, K], dt.float32, kind="ExternalInput")
    x_rows = nc.dram_tensor("x_rows", [N, M], dt.float32, kind="ExternalInput")
    out_rows = nc.dram_tensor("out_rows", [N, M], dt.float32,
                              kind="ExternalOutput")

    with TileContext(nc) as tc, ExitStack() as ctx:
        from concourse.library_config import mlp
        nc.gpsimd.load_library(mlp)
        const = ctx.enter_context(tc.tile_pool(name="const", bufs=1))
        sc = ctx.enter_context(tc.tile_pool(name="scratch", bufs=4))
        ep = ctx.enter_context(tc.tile_pool(name="epi", bufs=2))
        ps8 = ctx.enter_context(tc.tile_pool(name="ps8", bufs=2,
                                             space="PSUM"))

        tag_sb = const.tile([1, _tagw(kstage, ksub, rep_all)], dt.float32)
        nc.scalar.dma_start(tag_sb[:], reptag[:])
        xsb = const.tile([C + 1, NCH * 128], dt.bfloat16, name="xsb")

        for _rep in range(rep_all):
            _kernel_body(nc, tc, dt, mybir, kstage, ksub,
                         const, sc, ep, ps8, xsb,
                         xT1, w2cat, atiles, wkcol, x_rows, out_rows,
                         first=(_rep == 0), last=(_rep == rep_all - 1))

    nc.compile()
    return nc


def _kernel_body(nc, tc, dt, mybir, KSTAGE, KSUB,
                 const, sc, ep, ps8, xsb,
                 xT1, w2cat, atiles, wkcol, x_rows, out_rows,
                 first=True, last=True):
    AF = mybir.ActivationFunctionType
    ALU = mybir.AluOpType
    DR = mybir.MatmulPerfMode.DoubleRow

    # ---------------- loads ----------------
    # xT1 split per s-block: first MLP matmul can start ~3 us in.
    wq = SPC * 128
    if first:
        for s in range(NSB):
            nc.sync.dma_start(xsb[:, s * wq:(s + 1) * wq],
                              xT1[:, s * wq:(s + 1) * wq])
    w2_sb = sc.tile([C + 1, 2 * D], dt.bfloat16, tag="w2")
    nc.scalar.dma_start(w2_sb[:], w2cat[:])
    wk_sb = sc.tile([128, K], dt.float32, tag="wk")
    nc.scalar.dma_start(wk_sb[:], wkcol[:])
    bias_t = sc.tile([128, 1], dt.float32, tag="bias")
    nc.gpsimd.memset(bias_t[:], -LN8)

    # A^T slab: one 768 KB DMA per dst-tile (covers all 3 graphs).
    # Slabs 0-3 load upfront; the rest are JIT-prefetched inside the main
    # loop so the DMA device isn't saturated while epilogue I/O needs it.
    at_all = const.tile([128, NSB, K, NSB, 128], dt.float8e4)

    def load_at(ot):
        if "g" in KSUB:
            nc.sync.dma_start(
                at_all[:, ot].rearrange("p k s c -> p (k s c)"), atiles[ot])

    for ot in range(4):
        load_at(ot)
    if "g" not in KSUB:
        nc.gpsimd.memset(at_all[:], 0.125)

    # residual slabs prefetched into SBUF (avoids CCE-accumulate DMA parks
    # on the Pool queue); Pool does the add as a compute op instead.
    xr_tiles = {}

    def load_xr(ot):
        pv2 = 128 if ot < NSB - 1 else N - 128 * (NSB - 1)
        xr = ep.tile([128, M], dt.float32, tag="xr", bufs=4, name=f"xr{ot}")
        nc.sync.dma_start(xr[:pv2, :], x_rows[128 * ot:128 * ot + pv2, :])
        xr_tiles[ot] = xr

    # node tables: H[p, s, 0:384] = ES, H[p, s, 384:768] = G  (node 128s+p)
    # bufs=2 so rep i+1's prologue H writes overlap rep i's main loop
    # (matters for steady-state throughput when the body is replicated).
    H = ep.tile([128, NSB, 2 * M], dt.float8e4, tag="H", bufs=2, name="H")

    # ---------------- prologue: MLP -> H in SBUF, batched per s-block ------
    for s in range(NSB):
        sps = ps8.tile([128, M], dt.float32, tag="ps", bufs=8)
        fps = ps8.tile([128, M], dt.float32, tag="ps", bufs=8)
        for t in range(SPC):
            ci = s * SPC + t
            lt = xsb[:, 128 * ci:128 * (ci + 1)]
            nc.tensor.matmul(out=sps[:, D * t:D * (t + 1)], lhsT=lt,
                             rhs=w2_sb[:, D:], start=True, stop=True)
        for t in range(SPC):
            ci = s * SPC + t
            lt = xsb[:, 128 * ci:128 * (ci + 1)]
            nc.tensor.matmul(out=fps[:, D * t:D * (t + 1)], lhsT=lt,
                             rhs=w2_sb[:, :D], start=True, stop=True)
        esv = H[:, s, :M]
        nc.scalar.activation(esv, sps[:], AF.Exp, bias=bias_t[:])
        nc.vector.tensor_tensor(H[:, s, M:], fps[:], esv, op=ALU.mult)

    load_xr(0)
    load_xr(1)

    # Deferred per-ot tail: leaky (DVE) + residual (Pool) + store (SP),
    # emitted AFTER the next ot's k=0 drain ops so the DVE-queue park on
    # the Pool-produced sum never blocks a PSUM-draining recip/stt.
    def emit_tail(pend):
        ot0, pv0, acc0 = pend
        lr = ep.tile([128, M], dt.bfloat16, tag="lr", bufs=3, name="lr")
        nc.vector.scalar_tensor_tensor(lr[:pv0, :], acc0[:pv0, :], 0.01,
                                       acc0[:pv0, :], op0=ALU.mult,
                                       op1=ALU.max)
        ott = ep.tile([128, M], dt.float32, tag="out", bufs=3, name="ott")
        if KSTAGE >= "3":
            nc.gpsimd.tensor_tensor(ott[:pv0, :], lr[:pv0, :],
                                    xr_tiles[ot0][:pv0, :], op=ALU.add)
        else:
            nc.vector.tensor_copy(ott[:pv0, :], lr[:pv0, :])
        nc.sync.dma_start(out_rows[128 * ot0:128 * ot0 + pv0, :],
                          ott[:pv0, :])

    pend = None

    # ------------- main: [den|num] = A_k^T @ [ES|G], fused epilogue -------
    for ot in range(NSB if KSTAGE >= "1" else 0):
        pv = 128 if ot < NSB - 1 else N - 128 * (NSB - 1)
        if ot + 4 < NSB:
            load_at(ot + 4)
        if ot + 2 < NSB:
            load_xr(ot + 2)
        if ot == 8 and not last:
            # issue the NEXT rep's xsb loads now: by the rep boundary the
            # data is resident, so the next prologue starts with no PE gap
            # (and rides the warm pstate ramp).  WAR vs this rep's prologue
            # reads is long resolved.
            for s in range(NSB):
                nc.sync.dma_start(xsb[:, s * wq:(s + 1) * wq],
                                  xT1[:, s * wq:(s + 1) * wq])
        parts = []
        for k in range(K):
            if "m" not in KSUB:
                continue
            denp = ps8.tile([128, M], dt.float32, tag="ps", bufs=8)
            nump = ps8.tile([128, M], dt.float32, tag="ps", bufs=8)
            for s2 in range(NSB // 2):
                lt = at_all[:, ot, k, 2 * s2:2 * s2 + 2, :]
                nc.tensor.matmul(out=denp[:], lhsT=lt,
                                 rhs=H[:, 2 * s2:2 * s2 + 2, :M],
                                 start=(s2 == 0), stop=(s2 == NSB // 2 - 1),
                                 perf_mode=DR)
                nc.tensor.matmul(out=nump[:], lhsT=lt,
                                 rhs=H[:, 2 * s2:2 * s2 + 2, M:],
                                 start=(s2 == 0), stop=(s2 == NSB // 2 - 1),
                                 perf_mode=DR)
            if "e" not in KSUB:
                continue
            # epilogue part 1, all on DVE so the PSUM banks drain with zero
            # cross-engine hops: recip frees den; the fused stt
            # tmp_k = (num * w_k) * rden frees num.
            rdenb = ep.tile([128, M], dt.float32, tag="rden", bufs=3)
            nc.vector.reciprocal_approx_fast(out=rdenb[:pv, :],
                                             in_=denp[:pv, :])
            # Act drains num (folding w_k via Copy-with-scale) so DVE only
            # carries the recips + cheap bf16-side multiplies.
            numbw = ep.tile([128, M], dt.bfloat16, tag=f"nw{k}", bufs=2)
            nc.scalar.mul(numbw[:pv, :], nump[:pv, :], wk_sb[:pv, k:k + 1])
            tmp = ep.tile([128, M], dt.bfloat16, tag=f"tmp{k}", bufs=2)
            nc.vector.tensor_tensor(tmp[:pv, :], numbw[:pv, :],
                                    rdenb[:pv, :], op=ALU.mult)
            parts.append(tmp)
        if KSTAGE < "2" or not parts:
            continue
        # part 2: Pool pre-adds t0+t1 (ready one stt early); DVE finishes
        # with one add plus the fused leaky = max(x, .01x) stt.  Act stays
        # empty in the main loop.
        p01 = ep.tile([128, M], dt.bfloat16, tag="p01", bufs=2)
        nc.gpsimd.tensor_tensor(p01[:pv, :], parts[0][:pv, :],
                                parts[1][:pv, :], op=ALU.add)
        accv = ep.tile([128, M], dt.bfloat16, tag="acc", bufs=2)
        nc.gpsimd.tensor_tensor(accv[:pv, :], p01[:pv, :],
                                parts[2][:pv, :], op=ALU.add)
        emit_tail((ot, pv, accv))


def _resolve_kargs(kstage, ksub):
    import os
    if kstage is None:
        kstage = os.environ.get("KSTAGE", "3")
    if ksub is None:
        ksub = os.environ.get("KSUB", "gmeL")
    return kstage, ksub


def _tagw(kstage, ksub, rep_all):
    return 2 + rep_all * 8 + (ord(kstage) - ord("0")) + len(ksub) * 131


def _get_program(kstage=None, ksub=None, rep_all=1):
    key = ("nc", kstage, ksub, rep_all)
    if key not in _CACHE:
        _CACHE[key] = _build_program(kstage, ksub, rep_all)
    return _CACHE[key]


def _prep_inputs(input_feature, W_mlp, b_mlp, W_attn, b_attn, weight, edges):
    import ml_dtypes
    bf16 = ml_dtypes.bfloat16
    fp8 = ml_dtypes.float8_e4m3

    X = np.asarray(input_feature, dtype=np.float32)
    src = np.asarray(edges)[:, 0, :].astype(np.int64)
    dst = np.asarray(edges)[:, 1, :].astype(np.int64)
    assert src.min() >= 0 and src.max() < N
    assert dst.min() >= 0 and dst.max() < N

    A65 = np.concatenate([np.asarray(W_mlp, np.float32),
                          np.asarray(b_mlp, np.float32)[None, :]], axis=0)
    Wa = np.asarray(W_attn, np.float32)[:D, :]
    w2cat_h = np.ascontiguousarray(
        np.concatenate([A65, A65 @ Wa], axis=1).astype(bf16))  # [65, 128]

    # edge-count tiles: at_h[ot, p, k, s, c] = #edges(k, dst=128ot+c, src=128s+p)
    counts = np.zeros((K, NP, NP), np.uint8)
    kk = np.repeat(np.arange(K), E)
    np.add.at(counts, (kk, src.reshape(-1), dst.reshape(-1)), 1)
    at_h = np.ascontiguousarray(
        counts.reshape(K, NSB, 128, NSB, 128).transpose(3, 2, 0, 1, 4)
        .reshape(NSB, 128, K * NSB * 128).astype(fp8))

    wk = np.asarray(weight, np.float32).reshape(K)
    wkcol_h = np.ascontiguousarray(
        np.broadcast_to(wk[None, :], (128, K)).astype(np.float32))

    # per-core slices: slot = b*T + t; core c owns slots [6c, 6c+6)
    Xn = np.transpose(X, (1, 0, 2, 3)).reshape(N, BT, C)
    in_maps = []
    for c in range(NCORES):
        Xloc = Xn[:, SPC * c:SPC * (c + 1), :]                   # [N, 6, C]
        x_rows_h = np.ascontiguousarray(Xloc.reshape(N, M))
        Xpad = np.zeros((NP, SPC, C), np.float32)
        Xpad[:N] = Xloc
        xT1_h = np.empty((C + 1, NCH * 128), dtype=bf16)
        # col (s*SPC+t)*128 + i -> node 128s+i, slot t
        xT1_h[:C] = (Xpad.reshape(NSB, 128, SPC, C)
                     .transpose(3, 0, 2, 1).reshape(C, NCH * 128).astype(bf16))
        xT1_h[C] = np.asarray(1.0, dtype=bf16)
        in_maps.append({
            "xT1": np.ascontiguousarray(xT1_h),
            "w2cat": w2cat_h,
            "atiles": at_h,
            "wkcol": wkcol_h,
            "x_rows": x_rows_h,
        })
    return in_maps


def _assemble_output(results):
    out_all = np.empty((N, BT, C), dtype=np.float32)
    for c in range(NCORES):
        out_all[:, SPC * c:SPC * (c + 1), :] = \
            results[c]["out_rows"].reshape(N, SPC, C)
    return np.ascontiguousarray(
        out_all.reshape(N, B, T, C).transpose(1, 0, 2, 3))


def kernel(input_feature, W_mlp, b_mlp, W_attn, b_attn, weight, edges,
           _trace=False, **trace_kwargs):
    from concourse.bass_utils import run_bass_kernel_spmd

    in_maps = _prep_inputs(input_feature, W_mlp, b_mlp, W_attn, b_attn,
                           weight, edges)
    nc = _get_program()
    kstage, ksub = _resolve_kargs(None, None)
    tag = np.zeros((1, _tagw(kstage, ksub, 1)), np.float32)
    in_maps = [{**m, "reptag": tag} for m in in_maps]
    res = run_bass_kernel_spmd(nc, in_maps, list(range(NCORES)),
                               trace=_trace, **trace_kwargs)
    out = _assemble_output(res.results)
    if _trace:
        return out, res
    return out


# revision 45
# speedup vs baseline: 1.4119x; 1.0496x over previous
"""Trainium2 Bass kernel for nn_DilatedGCN (gnn_message_passing).

Math (derived from the reference):
  feats F = X @ W_mlp + b_mlp                  [N, B, T, D]
  scores = concat([F[src], F[dst]]) @ W_attn + b_attn
  Per-destination-segment softmax over the DEG=8 incoming edges.
  The dst-side term is constant within a segment, so it cancels in the
  softmax; max-subtraction is unnecessary in f32.  Hence with
     S  = F @ W_attn[:D]        (per node)
     ES = exp(S)/8              (per node; /8 keeps fp8e4 in range and
                                 cancels in num/den)
     G  = ES * F                (per node)
  each dilation graph k is a segment-sum over incoming edges:
     gcn_k[n] = (sum_j G[src_j]) / (sum_j ES[src_j])
  out = leaky_relu(sum_k w_k * gcn_k, 0.01) + X

Key idea vs the gather-based variant: dst = repeat(arange(N), 8), so
gather+segment-sum == dense matmul with the (tiny-valued, exact-in-fp8)
edge-count matrix A_k[dst, src]:
     [den | num] = A_k @ [ES | G]
A DMA row-gather is HBM-latency-bound; the dense matmul replaces it with
fp8 DoubleRow TensorE work plus 12 MB of sequential A-tile loads that
overlap compute.

Distribution: data-parallel over the 48 (b, t) pairs -> 6 per core.
Per core: the MLP runs per node-s-block (6 matmuls x 2 halves into two
PSUM banks, one batched Exp on Act, one batched mult on DVE) writing the
node tables H = [ES | G] (fp8, [128, 16 s-blocks, 768]) in SBUF; per
(ot, k) 8+8 DoubleRow matmuls (contraction 256/instr) produce den/num
in a single 8-bank rotating PSUM pool.  The epilogue drains PSUM with
minimal per-engine latency: DVE reciprocal_approx_fast frees den (the
bit-exact nc.vector.reciprocal is ~6 cycles/elem on HW -- never use it
here), Act frees num via Copy-with-scale folding w_k, DVE multiplies
num*rden in bf16, Pool sums the K=3 graph terms and adds the
SBUF-prefetched residual, leaky_relu(x,.01)=max(x,.01x) is one DVE stt,
stores go out on the SP queue.  A-slab and residual DMAs are
JIT-prefetched inside the ot loop so the DMA engines are never
saturated while epilogue I/O needs them; engine queues never carry DMAs
that would park in front of compute (parked DMAs block the whole
in-order queue).  HW-legality notes: Pool/GPSIMD cannot touch PSUM and
cannot run TensorScalarPtr; those constraints shaped the engine split.
"""

import os as _os
import shutil as _shutil

import numpy as np

# The libneuronxla on-disk NEFF cache keys on the XLA module NAME, not its
# content (libncc.py: cache_key = file_prefix.split('_')[-1]), so two
# different bass programs with the same jit callsite collide and the second
# silently runs the first one's stale NEFF. Disable the cache and clear any
# poisoned entries. One recompile is cheap; a wrong NEFF is not.
_shutil.rmtree(_os.path.expanduser("~/.neuron-compile-cache"),
               ignore_errors=True)

B, N, T, C, D, K, DEG = 4, 2000, 12, 64, 64, 3, 8
E = N * DEG
NCORES = 8
BT = B * T              # 48
SPC = BT // NCORES      # 6 (b,t) slots per core
M = SPC * D             # 384 channels per node per core
NSB = 16                # node s-blocks of 128 (2000 -> 2048 padded)
NP = 128 * NSB          # 2048 padded nodes
NCH = NSB * SPC         # 96 MLP chunks of 128 nodes x 1 slot
LN8 = float(np.log(8.0))

_CACHE = {}


def _build_program(kstage=None, ksub=None, rep_all=1):
    import concourse.bacc as bacc
    import concourse.mybir as mybir
    from concourse.tile import TileContext
    from contextlib import ExitStack

    kstage, ksub = _resolve_kargs(kstage, ksub)

    dt = mybir.dt
    nc = bacc.Bacc("TRN2")

    # Shape-encodes (rep_all, kstage, ksub) so every program variant gets a
    # distinct XLA fingerprint: the libneuronxla NEFF disk cache can collide
    # across programs whose external I/O signatures match, silently running
    # a stale NEFF. Consumed by a single tiny DMA so it can't be pruned.
    reptag = nc.dram_tensor("reptag", [1, _tagw(kstage, ksub, rep_all)],
                            dt.float32, kind="ExternalInput")
    xT1 = nc.dram_tensor("xT1", [C + 1, NCH * 128], dt.bfloat16,
                         kind="ExternalInput")
    w2cat = nc.dram_tensor("w2cat", [C + 1, 2 * D], dt.bfloat16,
                           kind="ExternalInput")
    # A^T tiles: atiles[ot, p, (k*16+s)*128 + c] = #edges(k, dst=128*ot+c,
    # src=128*s+p); counts <= 8 are exact in fp8e4.  ot-major so the first
    # dst-tile's lhsT data (all 3 graphs) lands in SBUF within ~2 us.
    atiles = nc.dram_tensor("atiles", [NSB, 128, K * NSB * 128], dt.float8e4,
                            kind="ExternalInput")
    wkcol = nc.dram_tensor("wkcol", [128, K], dt.float32, kind="ExternalInput")
    x_rows = nc.dram_tensor("x_rows", [N, M], dt.float32, kind="ExternalInput")
    out_rows = nc.dram_tensor("out_rows", [N, M], dt.float32,
                              kind="ExternalOutput")

    with TileContext(nc) as tc, ExitStack() as ctx:
        from concourse.library_config import mlp
        nc.gpsimd.load_library(mlp)
        const = ctx.enter_context(tc.tile_pool(name="const", bufs=1))
        sc = ctx.enter_context(tc.tile_pool(name="scratch", bufs=4))
        ep = ctx.enter_context(tc.tile_pool(name="epi", bufs=2))
        ps8 = ctx.enter_context(tc.tile_pool(name="ps8", bufs=2,
                                             space="PSUM"))

        tag_sb = const.tile([1, _tagw(kstage, ksub, rep_all)], dt.float32)
        nc.scalar.dma_start(tag_sb[:], reptag[:])
        xsb = const.tile([C + 1, NCH * 128], dt.bfloat16, name="xsb")

        for _rep in range(rep_all):
            _kernel_body(nc, tc, dt, mybir, kstage, ksub,
                         const, sc, ep, ps8, xsb,
                         xT1, w2cat, atiles, wkcol, x_rows, out_rows,
                         first=(_rep == 0), last=(_rep == rep_all - 1))

    nc.compile()
    return nc


def _kernel_body(nc, tc, dt, mybir, KSTAGE, KSUB,
                 const, sc, ep, ps8, xsb,
                 xT1, w2cat, atiles, wkcol, x_rows, out_rows,
                 first=True, last=True):
    AF = mybir.ActivationFunctionType
    ALU = mybir.AluOpType
    DR = mybir.MatmulPerfMode.DoubleRow

    # ---------------- loads ----------------
    # xT1 split per s-block: first MLP matmul can start ~3 us in.
    wq = SPC * 128
    if first:
        for s in range(NSB):
            nc.sync.dma_start(xsb[:, s * wq:(s + 1) * wq],
                              xT1[:, s * wq:(s + 1) * wq])
    w2_sb = sc.tile([C + 1, 2 * D], dt.bfloat16, tag="w2")
    nc.scalar.dma_start(w2_sb[:], w2cat[:])
    wk_sb = sc.tile([128, K], dt.float32, tag="wk")
    nc.scalar.dma_start(wk_sb[:], wkcol[:])
    bias_t = sc.tile([128, 1], dt.float32, tag="bias")
    nc.gpsimd.memset(bias_t[:], -LN8)

    # A^T slab: one 768 KB DMA per dst-tile (covers all 3 graphs).
    # Slabs 0-3 load upfront; the rest are JIT-prefetched inside the main
    # loop so the DMA device isn't saturated while epilogue I/O needs it.
    at_all = const.tile([128, NSB, K, NSB, 128], dt.float8e4)

    def load_at(ot):
        if "g" in KSUB:
            nc.sync.dma_start(
                at_all[:, ot].rearrange("p k s c -> p (k s c)"), atiles[ot])

    for ot in range(4):
        load_at(ot)
    if "g" not in KSUB:
        nc.gpsimd.memset(at_all[:], 0.125)

    # residual slabs prefetched into SBUF (avoids CCE-accumulate DMA parks
    # on the Pool queue); Pool does the add as a compute op instead.
    xr_tiles = {}

    def load_xr(ot):
        pv2 = 128 if ot < NSB - 1 else N - 128 * (NSB - 1)
        xr = ep.tile([128, M], dt.float32, tag="xr", bufs=4, name=f"xr{ot}")
        nc.sync.dma_start(xr[:pv2, :], x_rows[128 * ot:128 * ot + pv2, :])
        xr_tiles[ot] = xr

    # node tables: H[p, s, 0:384] = ES, H[p, s, 384:768] = G  (node 128s+p)
    # bufs=2 so rep i+1's prologue H writes overlap rep i's main loop
    # (matters for steady-state throughput when the body is replicated).
    H = ep.tile([128, NSB, 2 * M], dt.float8e4, tag="H", bufs=2, name="H")

    # ---------------- prologue: MLP -> H in SBUF, batched per s-block ------
    for s in range(NSB):
        sps = ps8.tile([128, M], dt.float32, tag="ps", bufs=8)
        fps = ps8.tile([128, M], dt.float32, tag="ps", bufs=8)
        for t in range(SPC):
            ci = s * SPC + t
            lt = xsb[:, 128 * ci:128 * (ci + 1)]
            nc.tensor.matmul(out=sps[:, D * t:D * (t + 1)], lhsT=lt,
                             rhs=w2_sb[:, D:], start=True, stop=True)
        for t in range(SPC):
            ci = s * SPC + t
            lt = xsb[:, 128 * ci:128 * (ci + 1)]
            nc.tensor.matmul(out=fps[:, D * t:D * (t + 1)], lhsT=lt,
                             rhs=w2_sb[:, :D], start=True, stop=True)
        esv = H[:, s, :M]
        nc.scalar.activation(esv, sps[:], AF.Exp, bias=bias_t[:])
        nc.vector.tensor_tensor(H[:, s, M:], fps[:], esv, op=ALU.mult)

    load_xr(0)
    load_xr(1)

    # Deferred per-ot tail: leaky (DVE) + residual (Pool) + store (SP),
    # emitted AFTER the next ot's k=0 drain ops so the DVE-queue park on
    # the Pool-produced sum never blocks a PSUM-draining recip/stt.
    def emit_tail(pend):
        ot0, pv0, acc0 = pend
        lr = ep.tile([128, M], dt.bfloat16, tag="lr", bufs=3, name="lr")
        if "L" in KSUB:
            # leaky on Act (SBUF-only read, so no accumulator-read tax;
            # Act idles in the main loop anyway).  CoreSim lacks Lrelu.
            nc.scalar.activation(lr[:pv0, :], acc0[:pv0, :], AF.Lrelu,
                                 alpha=0.01)
        else:
            nc.vector.scalar_tensor_tensor(lr[:pv0, :], acc0[:pv0, :], 0.01,
                                           acc0[:pv0, :], op0=ALU.mult,
                                           op1=ALU.max)
        ott = ep.tile([128, M], dt.float32, tag="out", bufs=3, name="ott")
        if KSTAGE >= "3":
            nc.gpsimd.tensor_tensor(ott[:pv0, :], lr[:pv0, :],
                                    xr_tiles[ot0][:pv0, :], op=ALU.add)
        else:
            nc.vector.tensor_copy(ott[:pv0, :], lr[:pv0, :])
        nc.sync.dma_start(out_rows[128 * ot0:128 * ot0 + pv0, :],
                          ott[:pv0, :])

    pend = None

    # ------------- main: [den|num] = A_k^T @ [ES|G], fused epilogue -------
    for ot in range(NSB if KSTAGE >= "1" else 0):
        pv = 128 if ot < NSB - 1 else N - 128 * (NSB - 1)
        if ot + 4 < NSB:
            load_at(ot + 4)
        if ot + 2 < NSB:
            load_xr(ot + 2)
        if ot == 8 and not last:
            # issue the NEXT rep's xsb loads now: by the rep boundary the
            # data is resident, so the next prologue starts with no PE gap
            # (and rides the warm pstate ramp).  WAR vs this rep's prologue
            # reads is long resolved.
            for s in range(NSB):
                nc.sync.dma_start(xsb[:, s * wq:(s + 1) * wq],
                                  xT1[:, s * wq:(s + 1) * wq])
        parts = []
        for k in range(K):
            if "m" not in KSUB:
                continue
            denp = ps8.tile([128, M], dt.float32, tag="ps", bufs=8)
            nump = ps8.tile([128, M], dt.float32, tag="ps", bufs=8)
            for s2 in range(NSB // 2):
                lt = at_all[:, ot, k, 2 * s2:2 * s2 + 2, :]
                nc.tensor.matmul(out=denp[:], lhsT=lt,
                                 rhs=H[:, 2 * s2:2 * s2 + 2, :M],
                                 start=(s2 == 0), stop=(s2 == NSB // 2 - 1),
                                 perf_mode=DR)
                nc.tensor.matmul(out=nump[:], lhsT=lt,
                                 rhs=H[:, 2 * s2:2 * s2 + 2, M:],
                                 start=(s2 == 0), stop=(s2 == NSB // 2 - 1),
                                 perf_mode=DR)
            if "e" not in KSUB:
                continue
            # epilogue part 1, all on DVE so the PSUM banks drain with zero
            # cross-engine hops: recip frees den; the fused stt
            # tmp_k = (num * w_k) * rden frees num.
            rdenb = ep.tile([128, M], dt.float32, tag="rden", bufs=3)
            nc.vector.reciprocal_approx_fast(out=rdenb[:pv, :],
                                             in_=denp[:pv, :])
            tmp = ep.tile([128, M], dt.bfloat16, tag=f"tmp{k}", bufs=2)
            nc.vector.scalar_tensor_tensor(tmp[:pv, :], nump[:pv, :],
                                           wk_sb[:pv, k:k + 1], rdenb[:pv, :],
                                           op0=ALU.mult, op1=ALU.mult)
            parts.append(tmp)
        if KSTAGE < "2" or not parts:
            continue
        # part 2: Pool pre-adds t0+t1 (ready one stt early); DVE finishes
        # with one add plus the fused leaky = max(x, .01x) stt.  Act stays
        # empty in the main loop.
        p01 = ep.tile([128, M], dt.bfloat16, tag="p01", bufs=2)
        nc.gpsimd.tensor_tensor(p01[:pv, :], parts[0][:pv, :],
                                parts[1][:pv, :], op=ALU.add)
        accv = ep.tile([128, M], dt.bfloat16, tag="acc", bufs=2)
        nc.gpsimd.tensor_tensor(accv[:pv, :], p01[:pv, :],
                                parts[2][:pv, :], op=ALU.add)
        emit_tail((ot, pv, accv))


def _resolve_kargs(kstage, ksub):
    import os
    if kstage is None:
        kstage = os.environ.get("KSTAGE", "3")
    if ksub is None:
        ksub = os.environ.get("KSUB", "gmeL")
    return kstage, ksub


def _tagw(kstage, ksub, rep_all):
    return 2 + rep_all * 8 + (ord(kstage) - ord("0")) + len(ksub) * 131


def _get_program(kstage=None, ksub=None, rep_all=1):
    key = ("nc", kstage, ksub, rep_all)
    if key not in _CACHE:
        _CACHE[key] = _build_program(kstage, ksub, rep_all)
    return _CACHE[key]


def _prep_inputs(input_feature, W_mlp, b_mlp, W_attn, b_attn, weight, edges):
    import ml_dtypes
    bf16 = ml_dtypes.bfloat16
    fp8 = ml_dtypes.float8_e4m3

    X = np.asarray(input_feature, dtype=np.float32)
    src = np.asarray(edges)[:, 0, :].astype(np.int64)
    dst = np.asarray(edges)[:, 1, :].astype(np.int64)
    assert src.min() >= 0 and src.max() < N
    assert dst.min() >= 0 and dst.max() < N

    A65 = np.concatenate([np.asarray(W_mlp, np.float32),
                          np.asarray(b_mlp, np.float32)[None, :]], axis=0)
    Wa = np.asarray(W_attn, np.float32)[:D, :]
    w2cat_h = np.ascontiguousarray(
        np.concatenate([A65, A65 @ Wa], axis=1).astype(bf16))  # [65, 128]

    # edge-count tiles: at_h[ot, p, k, s, c] = #edges(k, dst=128ot+c, src=128s+p)
    counts = np.zeros((K, NP, NP), np.uint8)
    kk = np.repeat(np.arange(K), E)
    np.add.at(counts, (kk, src.reshape(-1), dst.reshape(-1)), 1)
    at_h = np.ascontiguousarray(
        counts.reshape(K, NSB, 128, NSB, 128).transpose(3, 2, 0, 1, 4)
        .reshape(NSB, 128, K * NSB * 128).astype(fp8))

    wk = np.asarray(weight, np.float32).reshape(K)
    wkcol_h = np.ascontiguousarray(
        np.broadcast_to(wk[None, :], (128, K)).astype(np.float32))

    # per-core slices: slot = b*T + t; core c owns slots [6c, 6c+6)
    Xn = np.transpose(X, (1, 0, 2, 3)).reshape(N, BT, C)
    in_maps = []
    for c in range(NCORES):
        Xloc = Xn[:, SPC * c:SPC * (c + 1), :]                   # [N, 6, C]
        x_rows_h = np.ascontiguousarray(Xloc.reshape(N, M))
        Xpad = np.zeros((NP, SPC, C), np.float32)
        Xpad[:N] = Xloc
        xT1_h = np.empty((C + 1, NCH * 128), dtype=bf16)
        # col (s*SPC+t)*128 + i -> node 128s+i, slot t
        xT1_h[:C] = (Xpad.reshape(NSB, 128, SPC, C)
                     .transpose(3, 0, 2, 1).reshape(C, NCH * 128).astype(bf16))
        xT1_h[C] = np.asarray(1.0, dtype=bf16)
        in_maps.append({
            "xT1": np.ascontiguousarray(xT1_h),
            "w2cat": w2cat_h,
            "atiles": at_h,
            "wkcol": wkcol_h,
            "x_rows": x_rows_h,
        })
    return in_maps


def _assemble_output(results):
    out_all = np.empty((N, BT, C), dtype=np.float32)
    for c in range(NCORES):
        out_all[:, SPC * c:SPC * (c + 1), :] = \
            results[c]["out_rows"].reshape(N, SPC, C)
    return np.ascontiguousarray(
        out_all.reshape(N, B, T, C).transpose(1, 0, 2, 3))


def kernel(input_feature, W_mlp, b_mlp, W_attn, b_attn, weight, edges,
           _trace=False, **trace_kwargs):
    from concourse.bass_utils import run_bass_kernel_spmd

    in_maps = _prep_inputs(input_feature, W_mlp, b_mlp, W_attn, b_attn,
                           weight, edges)
    nc = _get_program()
    kstage, ksub = _resolve_kargs(None, None)
    tag = np.zeros((1, _tagw(kstage, ksub, 1)), np.float32)
    in_maps = [{**m, "reptag": tag} for m in in_maps]
    res = run_bass_kernel_spmd(nc, in_maps, list(range(NCORES)),
                               trace=_trace, **trace_kwargs)
    out = _assemble_output(res.results)
    if _trace:
        return out, res
    return out
